# revision 11
# baseline (speedup 1.0000x reference)
"""MoE (8 experts, top-2) Trainium2 kernel — fp8 DoubleRow edition.

Strategy: expert-parallel across 8 NeuronCores. The router (softmax ->
top-2 -> renormalize, ~0.03% of total FLOPs) runs on host in numpy; each
core runs one expert's gated FFN + residual + LayerNorm + combine-weight
scale over its assigned (capacity-padded) tokens. Host scatter-adds the
two weighted expert outputs per token.

All three matmuls run in fp8e4 (TRN E4M3, +-240) with
perf_mode=DoubleRow (2 fp8 weights/PE cell -> ~2x matmul throughput).
Power-of-2 scales keep the fp8 operands in the normal range:
  xt = x * 16, Wi/Wg/Wo = W * 1024, hT = h * 32
and every descale folds into constants that already exist:
  stage-1 ACT scale = 1/16384 (gelu), 0.5/16384 (tanh);
  th post-scale (16, 16) instead of (0.5, 0.5) makes hT = 32*h;
  xg/bo are host-prescaled by 32768 so the device epilogue is unchanged
  (LayerNorm normalization is scale-invariant; host combine divides by
  sqrt(var' + 32768^2 * eps), exact).

Device kernel (per core, SPMD), tokens in chunks of 512:
  stage 1: hT[f,c] = gelu(x@Wi + bi) * sigmoid(x@Wg + bg), F-major, fp8.
           DoubleRow over h-tile pairs (K=256/matmul, N=512 hides the
           DoubleRow LDWEIGHTS cost). sigmoid via tanh (same ACT table
           set as gelu -> no table reloads).
  stage 2: z = hT.T @ Wo in two H-half passes (PSUM: 4+4 banks);
           r = z + xg' + bo'; bn_stats/aggr; ship (r-mean)*w*gamma and
           (mean, var); host applies rsqrt + beta*w in the combine.
"""

import numpy as np
import ml_dtypes

E, TOPK, H, F = 8, 2, 1024, 4096
HT, FT = H // 128, F // 128
CHUNK = 512
EPS = 1e-12

SX = 16.0      # fp8 scale on x (tokens)
SW = 1024.0    # fp8 scale on Wi/Wg/Wo
SH = 32.0      # fp8 scale on hT (gated activations)
S1 = SX * SW   # stage-1 PSUM descale
KZ = SH * SW   # stage-2 z (and r/mean/var^0.5) scale

FP8 = ml_dtypes.float8_e4m3  # TRN fp8e4: max normal 240

_PROGRAM_CACHE: dict = {}


def _chunks_of(C: int):
    assert C % 128 == 0
    ch = [CHUNK] * (C // CHUNK)
    if C % CHUNK:
        ch.append(C % CHUNK)
    return ch


def _build_program(C: int, repeat: int = 1, sim_safe: bool = False):
    import concourse.mybir as mybir
    import concourse.tile as tile
    from concourse import bacc

    f32 = mybir.dt.float32
    fp8 = mybir.dt.float8e4
    ALU = mybir.AluOpType
    ACTF = mybir.ActivationFunctionType
    DR = mybir.MatmulPerfMode.DoubleRow
    # CoreSim doesn't implement Gelu; substitute an implemented LUT function
    # for simulator-only numerical checks.
    GELU_FUNC = ACTF.Sigmoid if sim_safe else ACTF.Gelu

    chunks = _chunks_of(C)
    NT = C // 128
    HP, FP = HT // 2, FT // 2  # h-tile pairs, f-tile pairs

    nc = bacc.Bacc("TRN2", target_bir_lowering=False, debug=False)

    wi_d = nc.dram_tensor("wi", [HT, 128, F], fp8, kind="ExternalInput")
    wg_d = nc.dram_tensor("wg", [HT, 128, F], fp8, kind="ExternalInput")
    wo_d = nc.dram_tensor("wo", [FT, 128, H], fp8, kind="ExternalInput")
    xt_d = nc.dram_tensor("xt", [128, HT * C], fp8, kind="ExternalInput")
    xg_d = nc.dram_tensor("xg", [NT, 128, H], f32, kind="ExternalInput")
    wt_d = nc.dram_tensor("wt", [128, NT], f32, kind="ExternalInput")
    bi_d = nc.dram_tensor("bi2", [128, FT], f32, kind="ExternalInput")
    bg_d = nc.dram_tensor("bg2", [128, FT], f32, kind="ExternalInput")
    bo_d = nc.dram_tensor("bo_r", [1, H], f32, kind="ExternalInput")
    gm_d = nc.dram_tensor("gm_r", [1, H], f32, kind="ExternalInput")
    y_d = nc.dram_tensor("y", [NT, 128, H], f32, kind="ExternalOutput")
    mv_d = nc.dram_tensor("mv", [NT, 128, 2], f32, kind="ExternalOutput")

    WSPLIT = 4  # column-split of weight loads so first tiles land early
    FS = F // WSPLIT

    with tile.TileContext(nc) as tc:
        with (
            tc.tile_pool(name="const", bufs=1) as constp,
            tc.tile_pool(name="wts", bufs=1) as wtsp,
            tc.tile_pool(name="xtp", bufs=2) as xtp,
            tc.tile_pool(name="htp", bufs=1) as htp,
            tc.tile_pool(name="tmp", bufs=3) as tmpp,
            tc.tile_pool(name="xgp", bufs=1) as xgp,
            tc.tile_pool(name="rp", bufs=2) as rp,
            tc.tile_pool(name="outp", bufs=2) as outp,
            tc.tile_pool(name="statp", bufs=2) as statp,
            tc.tile_pool(name="psA", bufs=2, space="PSUM") as psA,
            tc.tile_pool(name="psZ", bufs=1, space="PSUM") as psZ,
        ):
          for _rep in range(repeat):
            # small constants first (cheap, needed by first ACT/epilogue)
            bi_sb = constp.tile([128, FT], f32, tag="bi", name="bi_sb")
            nc.sync.dma_start(bi_sb, bi_d[:, :])
            bg_sb = constp.tile([128, FT], f32, tag="bg", name="bg_sb")
            nc.sync.dma_start(bg_sb, bg_d[:, :])
            wt_sb = constp.tile([128, NT], f32, tag="wt", name="wt_sb")
            nc.sync.dma_start(wt_sb, wt_d[:, :])
            bo_sb = constp.tile([128, H], f32, tag="bo", name="bo_sb")
            nc.sync.dma_start(bo_sb, bo_d.ap().to_broadcast((128, H)))
            gm_sb = constp.tile([128, H], f32, tag="gm", name="gm_sb")
            nc.sync.dma_start(gm_sb, gm_d.ap().to_broadcast((128, H)))

            wi_sb = wtsp.tile([128, HT, F], fp8, tag="wi", name="wi_sb")
            wg_sb = wtsp.tile([128, HT, F], fp8, tag="wg", name="wg_sb")
            wo_sb = wtsp.tile([128, FT, H], fp8, tag="wo", name="wo_sb")

            def load_xt(c0, CH):
                t = xtp.tile([128, HT, CH], fp8, tag="xt", name="xt_t")
                for h in range(HT):
                    nc.sync.dma_start(
                        t[:, h, :], xt_d[:, h * C + c0 : h * C + c0 + CH]
                    )
                return t

            xt_next = load_xt(0, chunks[0])
            c0 = 0
            for ch, CH in enumerate(chunks):
                NSUB = CH // 128
                xt_t = xt_next
                if ch == 0:
                    # weight loads, issued after chunk-0 activations and in
                    # f-column order so the first f-tiles' weights land first;
                    # wo is loaded once (SBUF-resident across chunks), after
                    # wi/wg since stage 2 starts ~a chunk later
                    for w in range(WSPLIT):
                        for h in range(HT):
                            nc.sync.dma_start(
                                wi_sb[:, h, w * FS : (w + 1) * FS],
                                wi_d[h, :, w * FS : (w + 1) * FS],
                            )
                            nc.sync.dma_start(
                                wg_sb[:, h, w * FS : (w + 1) * FS],
                                wg_d[h, :, w * FS : (w + 1) * FS],
                            )
                    for f in range(FT):
                        nc.sync.dma_start(wo_sb[:, f, :], wo_d[f])

                hT = htp.tile([128, FT, CH], fp8, tag="ht", name="hT")
                for f in range(FT):
                    fs = f * 128
                    ps_i = psA.tile([128, CH], f32, tag="psi", name="ps_i")
                    for t in range(HP):
                        nc.tensor.matmul(
                            ps_i,
                            wi_sb[:, 2 * t : 2 * t + 2, fs : fs + 128],
                            xt_t[:, 2 * t : 2 * t + 2, :],
                            start=(t == 0),
                            stop=(t == HP - 1),
                            perf_mode=DR,
                        )
                    gl = tmpp.tile([128, CH], f32, tag="gl", name="gl")
                    nc.scalar.activation(
                        gl, ps_i, GELU_FUNC, bias=bi_sb[:, f : f + 1],
                        scale=1.0 / S1,
                    )
                    ps_g = psA.tile([128, CH], f32, tag="psg", name="ps_g")
                    for t in range(HP):
                        nc.tensor.matmul(
                            ps_g,
                            wg_sb[:, 2 * t : 2 * t + 2, fs : fs + 128],
                            xt_t[:, 2 * t : 2 * t + 2, :],
                            start=(t == 0),
                            stop=(t == HP - 1),
                            perf_mode=DR,
                        )
                    # sigmoid(v) = 0.5*tanh(0.5*v)+0.5; hT = 32*h via
                    # th = 16*tanh + 16  (bg2 input is pre-scaled by 0.5)
                    th = tmpp.tile([128, CH], f32, tag="th", name="th")
                    nc.scalar.activation(
                        th, ps_g, ACTF.Tanh, bias=bg_sb[:, f : f + 1],
                        scale=0.5 / S1,
                    )
                    nc.vector.tensor_scalar(
                        th, th, SH / 2, SH / 2, op0=ALU.mult, op1=ALU.add
                    )
                    nc.vector.tensor_mul(hT[:, f, :], gl, th)

                if ch + 1 < len(chunks):
                    # prefetch next chunk's activations ahead of the wo stream
                    xt_next = load_xt(c0 + CH, chunks[ch + 1])

                # stage 2 in two H-half passes: 4 PSUM banks each pass,
                # coexists with stage-1's 4 psA banks
                rs_ = [
                    rp.tile([128, H], f32, tag=f"r{s}", name=f"r{s}")
                    for s in range(NSUB)
                ]
                xg_t = [None] * NSUB
                for s in range(NSUB):
                    st = c0 // 128 + s
                    xg_t[s] = xgp.tile([128, H], f32, tag=f"xg{s}", name="xg_t")
                    nc.sync.dma_start(xg_t[s], xg_d[st])
                for half in range(2):
                    hs = half * 512
                    zh = [
                        psZ.tile([128, 512], f32, tag=f"z{s}", name=f"zh{s}")
                        for s in range(NSUB)
                    ]
                    for t in range(FP):
                        for s in range(NSUB):
                            nc.tensor.matmul(
                                zh[s],
                                hT[:, 2 * t : 2 * t + 2, s * 128 : (s + 1) * 128],
                                wo_sb[:, 2 * t : 2 * t + 2, hs : hs + 512],
                                start=(t == 0),
                                stop=(t == FP - 1),
                                perf_mode=DR,
                            )
                    for s in range(NSUB):
                        nc.vector.tensor_add(
                            rs_[s][:, hs : hs + 512], zh[s],
                            xg_t[s][:, hs : hs + 512],
                        )

                for s in range(NSUB):
                    st = c0 // 128 + s
                    r = rs_[s]
                    nc.vector.tensor_add(r, r, bo_sb)
                    stats = statp.tile([128, 2, 6], f32, tag="stats", name="stats")
                    nc.vector.bn_stats(stats[:, 0, :], r[:, 0:512])
                    nc.vector.bn_stats(stats[:, 1, :], r[:, 512:H])
                    mv = statp.tile([128, 2], f32, tag="mv", name="mv")
                    nc.vector.bn_aggr(mv, stats)
                    # device ships (r - mean)*w*gamma and (mean, var); host
                    # multiplies by rsqrt(var + KZ^2*eps) and adds beta*w
                    # during the combine -> ACT never leaves the gelu/tanh
                    # table set (no table reloads).
                    nc.sync.dma_start(mv_d[st], mv)
                    nc.vector.tensor_scalar(
                        r, r, mv[:, 0:1], wt_sb[:, st : st + 1],
                        op0=ALU.subtract, op1=ALU.mult,
                    )
                    yt = outp.tile([128, H], f32, tag="y", name="yt")
                    nc.vector.tensor_mul(yt, r, gm_sb)
                    nc.sync.dma_start(y_d[st, :, 0:512], yt[:, 0:512])
                    nc.sync.dma_start(y_d[st, :, 512:H], yt[:, 512:H])
                c0 += CH

    nc.compile()
    return nc


def _get_program(C: int):
    if C not in _PROGRAM_CACHE:
        _PROGRAM_CACHE[C] = _build_program(C)
    return _PROGRAM_CACHE[C]


def _route(xf, Wr, br):
    """Replicates jax: softmax -> top_k(2) -> renormalize (fp32)."""
    logits = xf @ Wr + br
    m = logits.max(-1, keepdims=True)
    ex = np.exp(logits - m)
    probs = ex / ex.sum(-1, keepdims=True)
    topi = np.argsort(-probs, axis=-1, kind="stable")[:, :TOPK]
    topw = np.take_along_axis(probs, topi, -1)
    topw = topw / topw.sum(-1, keepdims=True)
    return topi, topw


def _q8(a, scale):
    return np.clip(a * np.float32(scale), -240, 240).astype(FP8)


def make_in_maps(x, Wr, br, Wi, bi, Wg, bg, Wo, bo, gamma, beta):
    """Shard inputs: route tokens, gather per-expert batches (padded to C)."""
    x = np.asarray(x, np.float32)
    B, S, _ = x.shape
    T = B * S
    xf = np.ascontiguousarray(x.reshape(T, H))
    topi, topw = _route(xf, np.asarray(Wr, np.float32), np.asarray(br, np.float32))

    idxs, wts = [], []
    for e in range(E):
        sel = np.nonzero((topi == e).any(-1))[0]
        w = topw[sel][topi[sel] == e]
        idxs.append(sel)
        wts.append(np.asarray(w, np.float32))
    cmax = max(len(s) for s in idxs)
    C = max(128, -(-cmax // 128) * 128)
    NT = C // 128

    Wi = np.asarray(Wi, np.float32)
    Wg = np.asarray(Wg, np.float32)
    Wo = np.asarray(Wo, np.float32)
    bi = np.asarray(bi, np.float32)
    bg = np.asarray(bg, np.float32)
    bo = np.asarray(bo, np.float32)
    gamma = np.asarray(gamma, np.float32)
    beta = np.asarray(beta, np.float32)

    def prep_expert(e):
        n = len(idxs[e])
        xg = np.zeros((C, H), np.float32)
        xg[:n] = xf[idxs[e]]
        wt = np.zeros((C,), np.float32)
        wt[:n] = wts[e]
        xtT = np.ascontiguousarray(xg.T)  # [H, C]
        xt8 = _q8(
            xtT.reshape(HT, 128, C).transpose(1, 0, 2).reshape(128, HT * C), SX
        )
        return {
            "wi": _q8(np.ascontiguousarray(Wi[e].reshape(HT, 128, F)), SW),
            "wg": _q8(np.ascontiguousarray(Wg[e].reshape(HT, 128, F)), SW),
            "wo": _q8(np.ascontiguousarray(Wo[e].reshape(FT, 128, H)), SW),
            "xt": np.ascontiguousarray(xt8),
            "xg": np.ascontiguousarray((xg * np.float32(KZ)).reshape(NT, 128, H)),
            "wt": np.ascontiguousarray(wt.reshape(NT, 128).T),
            "bi2": np.ascontiguousarray(bi[e].reshape(FT, 128).T),
            "bg2": np.ascontiguousarray((0.5 * bg[e]).reshape(FT, 128).T),
            "bo_r": np.ascontiguousarray((bo[e] * np.float32(KZ)).reshape(1, H)),
            "gm_r": np.ascontiguousarray(gamma[e].reshape(1, H)),
        }

    # numpy casts/copies release the GIL; threading cuts host prep ~4-8x
    from concurrent.futures import ThreadPoolExecutor

    with ThreadPoolExecutor(max_workers=E) as pool:
        in_maps = list(pool.map(prep_expert, range(E)))
    return in_maps, idxs, C, wts


def combine(results, idxs, C, T, wts=None, beta=None, out_dtype=np.float32):
    """Unshard: scatter-add weighted per-expert outputs back to tokens.

    The device ships A' = KZ*(r - mean)*w*gamma plus (KZ*mean, KZ^2*var);
    y = A' * rsqrt(KZ^2*var + KZ^2*eps) + beta*w is applied here (host
    fp32 rsqrt is exact, and keeping sqrt off the device avoids ACT table
    reloads).
    """
    eps_eff = np.float32(KZ * KZ * EPS)
    out = np.zeros((T, H), np.float32)
    for e in range(E):
        n = len(idxs[e])
        y = np.asarray(results[e]["y"], np.float32).reshape(C, H)[:n]
        var = np.asarray(results[e]["mv"], np.float32).reshape(C, 2)[:n, 1]
        rstd = 1.0 / np.sqrt(var + eps_eff)
        y = y * rstd[:, None]
        if beta is not None and wts is not None:
            y = y + np.outer(wts[e][:n], beta[e])
        out[idxs[e]] += y
    return out.astype(out_dtype)


def kernel(x, Wr, br, Wi, bi, Wg, bg, Wo, bo, gamma, beta):
    from concourse.bass_utils import run_bass_kernel_spmd

    x = np.asarray(x, np.float32)
    B, S, _ = x.shape
    in_maps, idxs, C, wts = make_in_maps(
        x, Wr, br, Wi, bi, Wg, bg, Wo, bo, gamma, beta
    )
    nc = _get_program(C)
    res = run_bass_kernel_spmd(nc, in_maps, list(range(E)))
    out = combine(
        res.results, idxs, C, B * S,
        wts=wts, beta=np.asarray(beta, np.float32),
    )
    return out.reshape(B, S, H)


# revision 12
# speedup vs baseline: 1.2747x; 1.2747x over previous
"""MoE (8 experts, top-2) Trainium2 kernel — fp8 DoubleRow edition.

Strategy: expert-parallel across 8 NeuronCores. The router (softmax ->
top-2 -> renormalize, ~0.03% of total FLOPs) runs on host in numpy; each
core runs one expert's gated FFN + residual + LayerNorm + combine-weight
scale over its assigned (capacity-padded) tokens. Host scatter-adds the
two weighted expert outputs per token.

All three matmuls run in fp8e4 (TRN E4M3, +-240) with
perf_mode=DoubleRow (2 fp8 weights/PE cell -> ~2x matmul throughput).
Power-of-2 scales keep the fp8 operands in the normal range:
  xt = x * 16, Wi/Wg/Wo = W * 1024, hT = h * 32
and every descale folds into constants that already exist:
  stage-1 ACT scale = 1/16384 (gelu), 0.5/16384 (tanh);
  th post-scale (16, 16) instead of (0.5, 0.5) makes hT = 32*h;
  xg/bo are host-prescaled by 32768 so the device epilogue is unchanged
  (LayerNorm normalization is scale-invariant; host combine divides by
  sqrt(var' + 32768^2 * eps), exact).

Device kernel (per core, SPMD), tokens in chunks of 512:
  stage 1: hT[f,c] = gelu(x@Wi + bi) * sigmoid(x@Wg + bg), F-major, fp8.
           DoubleRow over h-tile pairs (K=256/matmul, N=512 hides the
           DoubleRow LDWEIGHTS cost). sigmoid via tanh (same ACT table
           set as gelu -> no table reloads).
  stage 2: z = hT.T @ Wo in two H-half passes (PSUM: 4+4 banks);
           r = z + xg' + bo'; bn_stats/aggr; ship (r-mean)*w*gamma and
           (mean, var); host applies rsqrt + beta*w in the combine.
"""

import numpy as np
import ml_dtypes

E, TOPK, H, F = 8, 2, 1024, 4096
HT, FT = H // 128, F // 128
CHUNK = 384
EPS = 1e-12

SX = 16.0      # fp8 scale on x (tokens)
SW = 1024.0    # fp8 scale on Wi/Wg/Wo
SH = 32.0      # fp8 scale on hT (gated activations)
S1 = SX * SW   # stage-1 PSUM descale
KZ = SH * SW   # stage-2 z (and r/mean/var^0.5) scale

FP8 = ml_dtypes.float8_e4m3  # TRN fp8e4: max normal 240

_PROGRAM_CACHE: dict = {}


def _chunks_of(C: int):
    assert C % 128 == 0
    ch = [CHUNK] * (C // CHUNK)
    if C % CHUNK:
        ch.append(C % CHUNK)
    return ch


def _build_program(C: int, repeat: int = 1, sim_safe: bool = False):
    import concourse.mybir as mybir
    import concourse.tile as tile
    from concourse import bacc

    f32 = mybir.dt.float32
    fp8 = mybir.dt.float8e4
    ALU = mybir.AluOpType
    ACTF = mybir.ActivationFunctionType
    DR = mybir.MatmulPerfMode.DoubleRow
    # CoreSim doesn't implement Gelu; substitute an implemented LUT function
    # for simulator-only numerical checks.
    GELU_FUNC = ACTF.Sigmoid if sim_safe else ACTF.Gelu

    chunks = _chunks_of(C)
    NT = C // 128
    HP, FP = HT // 2, FT // 2  # h-tile pairs, f-tile pairs

    nc = bacc.Bacc("TRN2", target_bir_lowering=False, debug=False)

    wi_d = nc.dram_tensor("wi", [HT, 128, F], fp8, kind="ExternalInput")
    wg_d = nc.dram_tensor("wg", [HT, 128, F], fp8, kind="ExternalInput")
    wo_d = nc.dram_tensor("wo", [FT, 128, H], fp8, kind="ExternalInput")
    xt_d = nc.dram_tensor("xt", [128, HT * C], fp8, kind="ExternalInput")
    xg_d = nc.dram_tensor("xg", [NT, 128, H], f32, kind="ExternalInput")
    wt_d = nc.dram_tensor("wt", [128, NT], f32, kind="ExternalInput")
    bi_d = nc.dram_tensor("bi2", [128, FT], f32, kind="ExternalInput")
    bg_d = nc.dram_tensor("bg2", [128, FT], f32, kind="ExternalInput")
    bo_d = nc.dram_tensor("bo_r", [1, H], f32, kind="ExternalInput")
    gm_d = nc.dram_tensor("gm_r", [1, H], f32, kind="ExternalInput")
    y_d = nc.dram_tensor("y", [NT, 128, H], f32, kind="ExternalOutput")
    mv_d = nc.dram_tensor("mv", [NT, 128, 2], f32, kind="ExternalOutput")

    WSPLIT = 4  # column-split of weight loads so first tiles land early
    FS = F // WSPLIT

    with tile.TileContext(nc) as tc:
        with (
            tc.tile_pool(name="const", bufs=1) as constp,
            tc.tile_pool(name="wts", bufs=1) as wtsp,
            tc.tile_pool(name="xtp", bufs=2) as xtp,
            tc.tile_pool(name="htp", bufs=1) as htp,
            tc.tile_pool(name="tmp", bufs=3) as tmpp,
            tc.tile_pool(name="xgp", bufs=2) as xgp,
            tc.tile_pool(name="rp", bufs=2) as rp,
            tc.tile_pool(name="outp", bufs=2) as outp,
            tc.tile_pool(name="statp", bufs=2) as statp,
            tc.tile_pool(name="psA", bufs=2, space="PSUM") as psA,
            tc.tile_pool(name="psZ", bufs=1, space="PSUM") as psZ,
        ):
          for _rep in range(repeat):
            # small constants first (cheap, needed by first ACT/epilogue)
            bi_sb = constp.tile([128, FT], f32, tag="bi", name="bi_sb")
            nc.sync.dma_start(bi_sb, bi_d[:, :])
            bg_sb = constp.tile([128, FT], f32, tag="bg", name="bg_sb")
            nc.sync.dma_start(bg_sb, bg_d[:, :])
            wt_sb = constp.tile([128, NT], f32, tag="wt", name="wt_sb")
            nc.sync.dma_start(wt_sb, wt_d[:, :])
            bo_sb = constp.tile([128, H], f32, tag="bo", name="bo_sb")
            nc.sync.dma_start(bo_sb, bo_d.ap().to_broadcast((128, H)))
            gm_sb = constp.tile([128, H], f32, tag="gm", name="gm_sb")
            nc.sync.dma_start(gm_sb, gm_d.ap().to_broadcast((128, H)))

            wi_sb = wtsp.tile([128, HT, F], fp8, tag="wi", name="wi_sb")
            wg_sb = wtsp.tile([128, HT, F], fp8, tag="wg", name="wg_sb")
            wo_sb = wtsp.tile([128, FT, H], fp8, tag="wo", name="wo_sb")

            def load_xt(c0, CH):
                t = xtp.tile([128, HT, CH], fp8, tag="xt", name="xt_t")
                for h in range(HT):
                    nc.sync.dma_start(
                        t[:, h, :], xt_d[:, h * C + c0 : h * C + c0 + CH]
                    )
                return t

            xt_next = load_xt(0, chunks[0])
            c0 = 0
            for ch, CH in enumerate(chunks):
                NSUB = CH // 128
                xt_t = xt_next
                if ch == 0:
                    # weight loads, issued after chunk-0 activations and in
                    # f-column order so the first f-tiles' weights land first;
                    # wo is loaded once (SBUF-resident across chunks), after
                    # wi/wg since stage 2 starts ~a chunk later
                    for w in range(WSPLIT):
                        for h in range(HT):
                            nc.sync.dma_start(
                                wi_sb[:, h, w * FS : (w + 1) * FS],
                                wi_d[h, :, w * FS : (w + 1) * FS],
                            )
                            nc.sync.dma_start(
                                wg_sb[:, h, w * FS : (w + 1) * FS],
                                wg_d[h, :, w * FS : (w + 1) * FS],
                            )
                    for f in range(FT):
                        nc.sync.dma_start(wo_sb[:, f, :], wo_d[f])

                hT = htp.tile([128, FT, CH], fp8, tag="ht", name="hT")
                for f in range(FT):
                    fs = f * 128
                    ps_i = psA.tile([128, CH], f32, tag="psi", name="ps_i")
                    for t in range(HP):
                        nc.tensor.matmul(
                            ps_i,
                            wi_sb[:, 2 * t : 2 * t + 2, fs : fs + 128],
                            xt_t[:, 2 * t : 2 * t + 2, :],
                            start=(t == 0),
                            stop=(t == HP - 1),
                            perf_mode=DR,
                        )
                    gl = tmpp.tile([128, CH], f32, tag="gl", name="gl")
                    nc.scalar.activation(
                        gl, ps_i, GELU_FUNC, bias=bi_sb[:, f : f + 1],
                        scale=1.0 / S1,
                    )
                    ps_g = psA.tile([128, CH], f32, tag="psg", name="ps_g")
                    for t in range(HP):
                        nc.tensor.matmul(
                            ps_g,
                            wg_sb[:, 2 * t : 2 * t + 2, fs : fs + 128],
                            xt_t[:, 2 * t : 2 * t + 2, :],
                            start=(t == 0),
                            stop=(t == HP - 1),
                            perf_mode=DR,
                        )
                    # sigmoid(v) = 0.5*tanh(0.5*v)+0.5; hT = 32*h via
                    # th = 16*tanh + 16  (bg2 input is pre-scaled by 0.5)
                    th = tmpp.tile([128, CH], f32, tag="th", name="th")
                    nc.scalar.activation(
                        th, ps_g, ACTF.Tanh, bias=bg_sb[:, f : f + 1],
                        scale=0.5 / S1,
                    )
                    nc.vector.tensor_scalar(
                        th, th, SH / 2, SH / 2, op0=ALU.mult, op1=ALU.add
                    )
                    nc.vector.tensor_mul(hT[:, f, :], gl, th)

                if ch + 1 < len(chunks):
                    # prefetch next chunk's activations ahead of the wo stream
                    xt_next = load_xt(c0 + CH, chunks[ch + 1])

                # stage 2 in two H-half passes: 4 PSUM banks each pass,
                # coexists with stage-1's 4 psA banks
                rs_ = [
                    rp.tile([128, H], f32, tag=f"r{s}", name=f"r{s}")
                    for s in range(NSUB)
                ]
                xg_t = [None] * NSUB
                for s in range(NSUB):
                    st = c0 // 128 + s
                    xg_t[s] = xgp.tile([128, H], f32, tag=f"xg{s}", name="xg_t")
                    nc.sync.dma_start(xg_t[s], xg_d[st])
                for half in range(2):
                    hs = half * 512
                    zh = [
                        psZ.tile([128, 512], f32, tag=f"z{s}", name=f"zh{s}")
                        for s in range(NSUB)
                    ]
                    for t in range(FP):
                        for s in range(NSUB):
                            nc.tensor.matmul(
                                zh[s],
                                hT[:, 2 * t : 2 * t + 2, s * 128 : (s + 1) * 128],
                                wo_sb[:, 2 * t : 2 * t + 2, hs : hs + 512],
                                start=(t == 0),
                                stop=(t == FP - 1),
                                perf_mode=DR,
                            )
                    for s in range(NSUB):
                        nc.vector.tensor_add(
                            rs_[s][:, hs : hs + 512], zh[s],
                            xg_t[s][:, hs : hs + 512],
                        )

                for s in range(NSUB):
                    st = c0 // 128 + s
                    r = rs_[s]
                    nc.vector.tensor_add(r, r, bo_sb)
                    stats = statp.tile([128, 2, 6], f32, tag="stats", name="stats")
                    nc.vector.bn_stats(stats[:, 0, :], r[:, 0:512])
                    nc.vector.bn_stats(stats[:, 1, :], r[:, 512:H])
                    mv = statp.tile([128, 2], f32, tag="mv", name="mv")
                    nc.vector.bn_aggr(mv, stats)
                    # device ships (r - mean)*w*gamma and (mean, var); host
                    # multiplies by rsqrt(var + KZ^2*eps) and adds beta*w
                    # during the combine -> ACT never leaves the gelu/tanh
                    # table set (no table reloads).
                    nc.sync.dma_start(mv_d[st], mv)
                    nc.vector.tensor_scalar(
                        r, r, mv[:, 0:1], wt_sb[:, st : st + 1],
                        op0=ALU.subtract, op1=ALU.mult,
                    )
                    yt = outp.tile([128, H], f32, tag="y", name="yt")
                    nc.vector.tensor_mul(yt, r, gm_sb)
                    nc.sync.dma_start(y_d[st, :, 0:512], yt[:, 0:512])
                    nc.sync.dma_start(y_d[st, :, 512:H], yt[:, 512:H])
                c0 += CH

    nc.compile()
    return nc


def _get_program(C: int):
    if C not in _PROGRAM_CACHE:
        _PROGRAM_CACHE[C] = _build_program(C)
    return _PROGRAM_CACHE[C]


def _route(xf, Wr, br):
    """Replicates jax: softmax -> top_k(2) -> renormalize (fp32)."""
    logits = xf @ Wr + br
    m = logits.max(-1, keepdims=True)
    ex = np.exp(logits - m)
    probs = ex / ex.sum(-1, keepdims=True)
    topi = np.argsort(-probs, axis=-1, kind="stable")[:, :TOPK]
    topw = np.take_along_axis(probs, topi, -1)
    topw = topw / topw.sum(-1, keepdims=True)
    return topi, topw


def _q8(a, scale):
    return np.clip(a * np.float32(scale), -240, 240).astype(FP8)


def make_in_maps(x, Wr, br, Wi, bi, Wg, bg, Wo, bo, gamma, beta):
    """Shard inputs: route tokens, gather per-expert batches (padded to C)."""
    x = np.asarray(x, np.float32)
    B, S, _ = x.shape
    T = B * S
    xf = np.ascontiguousarray(x.reshape(T, H))
    topi, topw = _route(xf, np.asarray(Wr, np.float32), np.asarray(br, np.float32))

    idxs, wts = [], []
    for e in range(E):
        sel = np.nonzero((topi == e).any(-1))[0]
        w = topw[sel][topi[sel] == e]
        idxs.append(sel)
        wts.append(np.asarray(w, np.float32))
    cmax = max(len(s) for s in idxs)
    C = max(128, -(-cmax // 128) * 128)
    NT = C // 128

    Wi = np.asarray(Wi, np.float32)
    Wg = np.asarray(Wg, np.float32)
    Wo = np.asarray(Wo, np.float32)
    bi = np.asarray(bi, np.float32)
    bg = np.asarray(bg, np.float32)
    bo = np.asarray(bo, np.float32)
    gamma = np.asarray(gamma, np.float32)
    beta = np.asarray(beta, np.float32)

    def prep_expert(e):
        n = len(idxs[e])
        xg = np.zeros((C, H), np.float32)
        xg[:n] = xf[idxs[e]]
        wt = np.zeros((C,), np.float32)
        wt[:n] = wts[e]
        xtT = np.ascontiguousarray(xg.T)  # [H, C]
        xt8 = _q8(
            xtT.reshape(HT, 128, C).transpose(1, 0, 2).reshape(128, HT * C), SX
        )
        return {
            "wi": _q8(np.ascontiguousarray(Wi[e].reshape(HT, 128, F)), SW),
            "wg": _q8(np.ascontiguousarray(Wg[e].reshape(HT, 128, F)), SW),
            "wo": _q8(np.ascontiguousarray(Wo[e].reshape(FT, 128, H)), SW),
            "xt": np.ascontiguousarray(xt8),
            "xg": np.ascontiguousarray((xg * np.float32(KZ)).reshape(NT, 128, H)),
            "wt": np.ascontiguousarray(wt.reshape(NT, 128).T),
            "bi2": np.ascontiguousarray(bi[e].reshape(FT, 128).T),
            "bg2": np.ascontiguousarray((0.5 * bg[e]).reshape(FT, 128).T),
            "bo_r": np.ascontiguousarray((bo[e] * np.float32(KZ)).reshape(1, H)),
            "gm_r": np.ascontiguousarray(gamma[e].reshape(1, H)),
        }

    # numpy casts/copies release the GIL; threading cuts host prep ~4-8x
    from concurrent.futures import ThreadPoolExecutor

    with ThreadPoolExecutor(max_workers=E) as pool:
        in_maps = list(pool.map(prep_expert, range(E)))
    return in_maps, idxs, C, wts


def combine(results, idxs, C, T, wts=None, beta=None, out_dtype=np.float32):
    """Unshard: scatter-add weighted per-expert outputs back to tokens.

    The device ships A' = KZ*(r - mean)*w*gamma plus (KZ*mean, KZ^2*var);
    y = A' * rsqrt(KZ^2*var + KZ^2*eps) + beta*w is applied here (host
    fp32 rsqrt is exact, and keeping sqrt off the device avoids ACT table
    reloads).
    """
    eps_eff = np.float32(KZ * KZ * EPS)
    out = np.zeros((T, H), np.float32)
    for e in range(E):
        n = len(idxs[e])
        y = np.asarray(results[e]["y"], np.float32).reshape(C, H)[:n]
        var = np.asarray(results[e]["mv"], np.float32).reshape(C, 2)[:n, 1]
        rstd = 1.0 / np.sqrt(var + eps_eff)
        y = y * rstd[:, None]
        if beta is not None and wts is not None:
            y = y + np.outer(wts[e][:n], beta[e])
        out[idxs[e]] += y
    return out.astype(out_dtype)


def kernel(x, Wr, br, Wi, bi, Wg, bg, Wo, bo, gamma, beta):
    from concourse.bass_utils import run_bass_kernel_spmd

    x = np.asarray(x, np.float32)
    B, S, _ = x.shape
    in_maps, idxs, C, wts = make_in_maps(
        x, Wr, br, Wi, bi, Wg, bg, Wo, bo, gamma, beta
    )
    nc = _get_program(C)
    res = run_bass_kernel_spmd(nc, in_maps, list(range(E)))
    out = combine(
        res.results, idxs, C, B * S,
        wts=wts, beta=np.asarray(beta, np.float32),
    )
    return out.reshape(B, S, H)
